# revision 67
# baseline (speedup 1.0000x reference)
"""Trainium2 Bass kernel for nn_Attention3d (3D attention with maxpooled K/V).

Reference computation per sample b:
    xf = x[b].reshape(C, Nq)                    C=128, Nq=24^3=13824
    q  = w_q @ xf                               [16, Nq]
    k  = maxpool2(w_k conv x)   -> [16, Nk]     Nk=12^3=1728
    v  = maxpool2(w_v conv x)   -> [64, Nk]
    attn = softmax_over_k(k^T q)                [Nk, Nq]
    o  = v @ attn                               [64, Nq]
    out = gamma * (w_o @ o) + xf

Sharding: data-parallel over batch B=8 -> 8 NeuronCores, one sample each.

Kernel structure (per core):
  1. conv phase: QKV 1x1 convs as matmuls (float32r, full rate)
  2. maxpool on DVE (3 in-place strided max stages)
  3. K replicated to 3 partition bases (row-packed S matmuls), V transposed
     via TensorE into V_T tiles with an appended ones-column (so the O matmul
     also produces the softmax denominator row for free)
  4. main loop over 27 query chunks of 512:
       S = K^T Q (bf16, 3-way row-tiled), exp on ScalarE (PSUM->SBUF, bf16),
       O = V_aug^T E (bf16 accumulation over 14 key tiles)
  5. denominator: reciprocal * gamma
  6. output loop: Y = w_o @ O (f32r), scale by broadcast 1/D, add residual, DMA out
"""

import numpy as np
from contextlib import ExitStack

import concourse.bacc as bacc
import concourse.bass as bass
import concourse.tile as tile
from concourse import mybir
from concourse.masks import make_identity

F32 = mybir.dt.float32
F32R = mybir.dt.float32r
BF16 = mybir.dt.bfloat16

C = 128
CA = 16
CV = 64
LL = 24
NQ = LL * LL * LL          # 13824
NKR = (LL // 2) ** 3       # 1728 real keys
NKT = 14                   # key tiles of 128 (last padded 64)
NKP = NKT * 128            # 1792
CHW = 512                  # query chunk width
NCH = NQ // CHW            # 27


def r32(ap):
    return ap.bitcast(F32R)


def build_program():
    nc = bacc.Bacc("TRN2", target_bir_lowering=False, debug=False, num_devices=8)

    x_d = nc.dram_tensor("x", [C, NQ], F32, kind="ExternalInput")
    wq_d = nc.dram_tensor("w_q", [CA, C], F32, kind="ExternalInput")
    wk_d = nc.dram_tensor("w_k", [CA, C], F32, kind="ExternalInput")
    wv_d = nc.dram_tensor("w_v", [CV, C], F32, kind="ExternalInput")
    wo_d = nc.dram_tensor("w_o", [C, CV], F32, kind="ExternalInput")
    g_d = nc.dram_tensor("gamma", [1, 1], F32, kind="ExternalInput")
    out_d = nc.dram_tensor("out", [C, NQ], F32, kind="ExternalOutput")
    dscr_d = nc.dram_tensor("dscr", [NCH, CHW], F32, kind="Internal")

    with tile.TileContext(nc) as tc, ExitStack() as ctx:
        singles = ctx.enter_context(tc.tile_pool(name="singles", bufs=1))
        big = ctx.enter_context(tc.tile_pool(name="big", bufs=1))
        e_pool = ctx.enter_context(tc.tile_pool(name="epool", bufs=5))
        r_pool = ctx.enter_context(tc.tile_pool(name="rpool", bufs=3))
        out_pool = ctx.enter_context(tc.tile_pool(name="outpool", bufs=3))

        # ---------------- constants / weights ----------------
        # lhsT for K/V conv: columns 0:64 = w_v^T (V on psum rows 0:64, base-0
        # for the PE transpose), columns 64:80 = w_k^T (K on rows 64:80)
        lhsT_kv = singles.tile([C, 80], F32)
        nc.sync.dma_start(lhsT_kv[:, 0:64], wv_d.ap().transpose([1, 0]))
        nc.sync.dma_start(lhsT_kv[:, 64:80], wk_d.ap().transpose([1, 0]))
        # lhsT for Q conv: w_q^T replicated at columns 0/32/64 so the conv
        # emits Q at partition bases 0/32/64 directly (for row-tiled S matmuls)
        lhsT_q = singles.tile([C, 80], F32)
        nc.vector.memset(lhsT_q[:, :], 0.0)
        for base in (0, 32, 64):
            nc.sync.dma_start(lhsT_q[:, base:base + CA],
                              wq_d.ap().transpose([1, 0]))
        # w_o^T as [64, 128] bf16 (lhsT of the Y matmul)
        w_oT_f = singles.tile([CV, C], F32)
        nc.sync.dma_start(w_oT_f[:, :], wo_d.ap().transpose([1, 0]))
        w_oT = singles.tile([CV, C], BF16)
        nc.vector.tensor_copy(w_oT[:, :], w_oT_f[:, :])
        # gamma broadcast down a column for the per-partition scalar multiply
        gamma_col = singles.tile([C, 1], F32)
        nc.sync.dma_start(gamma_col[:, :], g_d.ap().to_broadcast((C, 1)))
        # identity for PE transpose (bf16 to match kv_sb)
        ident = singles.tile([C, C], BF16)
        make_identity(nc, ident[:, :])

        # ---------------- big SBUF buffers ----------------
        x_pool = ctx.enter_context(tc.tile_pool(name="xstage", bufs=3))
        q_rep = big.tile([96, NQ], BF16)          # Q at partition bases 0/32/64
        o_buf = big.tile([CV, NQ], BF16)          # O rows 0:64 (bf16 for Y matmul)
        d_pool = ctx.enter_context(tc.tile_pool(name="dpool", bufs=3))
        k_rep = singles.tile([96, NKP], BF16)     # pooled K at bases 0/32/64, padded
        v_taug = singles.tile([C, NKT * (CV + 1)], BF16)  # V^T tiles + ones column
        # denominator blocks of 4 chunks (base-0 tiles, folded incrementally)
        DBLK = 4
        ndb = (NCH + DBLK - 1) // DBLK
        dm_tiles = [singles.tile([min(DBLK, NCH - k * DBLK), CHW], F32,
                                 name=f"dm{k}") for k in range(ndb)]

        with tc.tile_pool(name="kvbuf", bufs=1) as kv_pool:
            kv_sb = kv_pool.tile([80, NQ], BF16)  # V/K conv out; pooled in place

            # ---------------- phase 1: conv (f32r = full-rate PE) ----------
            lhsT_kv_r = singles.tile([C, 80], F32R)
            nc.vector.tensor_copy(lhsT_kv_r[:, :], lhsT_kv[:, :])
            lhsT_q_r = singles.tile([C, 80], F32R)
            nc.vector.tensor_copy(lhsT_q_r[:, :], lhsT_q[:, :])
            with tc.tile_pool(name="convps", bufs=4, space="PSUM") as conv_ps:
                BLK = 2 * CHW  # 1024-wide input DMA blocks (fewer DMA issues)
                kvt = kv_sb[:, :]

                def kv_strided(off, dims):
                    return bass.AP(
                        tensor=kvt.tensor, offset=kvt.offset + off,
                        ap=[list(kvt.ap[0])] + [[s, c] for s, c in dims])

                for bi, blk in enumerate(range(0, NQ, BLK)):
                    bw = min(BLK, NQ - blk)
                    xs = x_pool.tile([C, BLK], F32, tag="xs")
                    dma_eng = (nc.sync, nc.gpsimd)[bi % 2]
                    dma_eng.dma_start(xs[:, 0:bw], x_d.ap()[:, blk:blk + bw])
                    xr = x_pool.tile([C, BLK], F32R, tag="xr")
                    nc.vector.tensor_copy(xr[:, 0:bw], xs[:, 0:bw])
                    for qi, q0 in enumerate(range(0, bw, CHW)):
                        sl = slice(blk + q0, blk + q0 + CHW)
                        cps = conv_ps.tile([80, CHW], F32, tag="cps")
                        nc.tensor.matmul(cps[:, :], lhsT_kv_r[:, :],
                                         xr[:, q0:q0 + CHW],
                                         start=True, stop=True)
                        qps = conv_ps.tile([80, CHW], F32, tag="cps")
                        nc.tensor.matmul(qps[:, :], lhsT_q_r[:, :],
                                         xr[:, q0:q0 + CHW],
                                         start=True, stop=True)
                        # KV copy on ScalarE (idle until first exp);
                        # alternate Q copies between DVE and ScalarE
                        nc.scalar.copy(kv_sb[:, sl], cps[:, :])
                        if qi % 2 == 0:
                            nc.vector.tensor_copy(q_rep[0:80, sl], qps[:, :])
                        else:
                            nc.scalar.copy(q_rep[0:80, sl], qps[:, :])
                    # maxpool stage 1 (w-pairs) for this block, in place:
                    # reads kv[blk : blk+bw], writes kv[blk/2 : blk/2+bw/2]
                    nc.vector.tensor_max(
                        kv_strided(blk // 2, [(1, bw // 2)]),
                        kv_strided(blk, [(2, bw // 2)]),
                        kv_strided(blk + 1, [(2, bw // 2)]))


            # ------------- phase 2: maxpool stages 2+3 (in place) -----------
            # kv_sb rows: 0:64 V, 64:80 K. After stage 1: idx = l*288+h*12+w2.
            # stage 2: pairs along h -> [80, 24*12*12]
            a2 = kv_strided(0, [(288, 24), (24, 12), (1, 12)])
            b2 = kv_strided(12, [(288, 24), (24, 12), (1, 12)])
            o2 = kv_strided(0, [(144, 24), (12, 12), (1, 12)])
            nc.vector.tensor_max(o2, a2, b2)
            # stage 3: pairs along l -> [80, 12^3]: pooled K/V in kv_sb[:, 0:1728]
            a3 = kv_strided(0, [(288, 12), (12, 12), (1, 12)])
            b3 = kv_strided(144, [(288, 12), (12, 12), (1, 12)])
            o3 = kv_strided(0, [(144, 12), (12, 12), (1, 12)])
            nc.vector.tensor_max(o3, a3, b3)

            # ---------- phase 3: K replication + V transpose ----------
            nc.gpsimd.memset(k_rep[:, :], 0.0)
            nc.vector.tensor_copy(k_rep[64:64 + CA, 0:NKR], kv_sb[64:80, 0:NKR])
            nc.sync.dma_start(k_rep[0:CA, :], k_rep[64:64 + CA, :])
            nc.sync.dma_start(k_rep[32:32 + CA, :], k_rep[64:64 + CA, :])

            nc.gpsimd.memset(v_taug[:, :], 0.0)
            with tc.tile_pool(name="tpps", bufs=2, space="PSUM") as tp_pool:
                for t in range(NKT):
                    cols = 128 if t < NKT - 1 else NKR - 128 * (NKT - 1)  # 64 last
                    tp = tp_pool.tile([C, CV], BF16, tag="tp")
                    nc.tensor.transpose(tp[0:cols, :],
                                        kv_sb[0:64, t * 128:t * 128 + cols],
                                        ident[0:64, 0:64])
                    nc.scalar.copy(v_taug[0:cols, t * 65:t * 65 + CV],
                                   tp[0:cols, :])
                    nc.gpsimd.memset(v_taug[0:cols, t * 65 + CV:t * 65 + CV + 1],
                                     1.0)

        # ---------------- phases 4-6: attention + output, one pipeline ------
        # Flat stream of (chunk, key-tile-group) units, software-pipelined so
        # ScalarE (exp, the bottleneck) runs back-to-back.  The output phase
        # for chunk c is interleaved once chunk c's denominator is ready
        # (denominators are folded in two halves).
        sps_pool = ctx.enter_context(tc.tile_pool(name="sps", bufs=2, space="PSUM"))
        ps_small = ctx.enter_context(tc.tile_pool(name="pssm", bufs=2, space="PSUM"))
        GROUPS = [(0, 3), (3, 3), (6, 3), (9, 3), (12, 2)]  # (tile0, ntiles)
        NG = len(GROUPS)
        HALF = 14  # denominator fold boundary (chunks 0:HALF, HALF:NCH)

        units = [(ch, g) for ch in range(NCH) for g in range(NG)]
        NU = len(units)
        s_tiles = {}
        e_tiles = {}
        o_tiles = {}

        def s_group(u):
            ch, g = units[u]
            g0, gn = GROUPS[g]
            sl = bass.ts(ch, CHW)
            s_ps = sps_pool.tile([C, 3 * CHW], F32, tag="sps")
            s_tiles[u] = s_ps
            for t in range(g0, g0 + gn):
                j = t % 3
                nc.tensor.matmul(
                    s_ps[:, (t - g0) * CHW:(t - g0 + 1) * CHW],
                    k_rep[32 * j:32 * j + CA, t * 128:(t + 1) * 128],
                    q_rep[32 * j:32 * j + CA, sl],
                    start=True, stop=True,
                    tile_position=(32 * j, 0),
                )

        def exp_group(u):
            ch, g = units[u]
            g0, gn = GROUPS[g]
            et = e_pool.tile([C, 3 * CHW], BF16)
            nc.scalar.activation(et[:, 0:gn * CHW],
                                 s_tiles[u][:, 0:gn * CHW],
                                 mybir.ActivationFunctionType.Exp)
            e_tiles[u] = et
            del s_tiles[u]

        def fold_denominator(dm, lo, hi):
            n = hi - lo
            nc.vector.reciprocal(dm[0:n, :], dm[0:n, :])
            nc.vector.tensor_scalar_mul(dm[0:n, :], dm[0:n, :],
                                        gamma_col[0:n, :])
            nc.sync.dma_start(dscr_d.ap()[lo:hi, :], dm[0:n, :])

        def phase6_pair(c, ncc):
            """Output chunks c .. c+ncc-1 (ncc in {1,2}); paired DMAs."""
            w = ncc * CHW
            lo, hi = c * CHW, c * CHW + w
            r_sb = r_pool.tile([C, 2 * CHW], F32, name="r_sb")
            nc.gpsimd.dma_start(
                r_sb[:, 0:w],
                bass.AP(tensor=dscr_d.ap().tensor, offset=c * CHW,
                        ap=[[0, C], [1, w]]))
            xs6 = x_pool.tile([C, 2 * CHW], F32, tag="xs", name="xs6")
            nc.gpsimd.dma_start(xs6[:, 0:w], x_d.ap()[:, lo:hi])
            ot = out_pool.tile([C, 2 * CHW], F32, name="ot")
            for i in range(ncc):
                y_ps = ps_small.tile([C, CHW], F32, tag="ps", name="y_ps")
                nc.tensor.matmul(y_ps[:, :], w_oT[:, :],
                                 o_buf[0:CV, bass.ts(c + i, CHW)],
                                 start=True, stop=True)
                nc.vector.tensor_tensor(ot[:, bass.ts(i, CHW)], y_ps[:, :],
                                        r_sb[:, bass.ts(i, CHW)],
                                        mybir.AluOpType.mult)
            nc.vector.tensor_tensor(ot[:, 0:w], ot[:, 0:w], xs6[:, 0:w],
                                    mybir.AluOpType.add)
            nc.gpsimd.dma_start(out_d.ap()[:, lo:hi], ot[:, 0:w])

        def o_group(u):
            ch, g = units[u]
            g0, gn = GROUPS[g]
            sl = bass.ts(ch, CHW)
            if g == 0:
                o_tiles[ch] = ps_small.tile([CV + 1, CHW], F32, tag="ps",
                                            name="o_ps")
            o_ps = o_tiles[ch]
            et = e_tiles[u]
            for t in range(g0, g0 + gn):
                nc.tensor.matmul(
                    o_ps[:, :],
                    v_taug[:, t * 65:(t + 1) * 65],
                    et[:, (t - g0) * CHW:(t - g0 + 1) * CHW],
                    start=(t == 0), stop=(t == NKT - 1),
                )
            del e_tiles[u]
            if g == NG - 1:
                # chunk complete: export O and its denominator row
                nc.vector.tensor_copy(o_buf[:, sl], o_ps[0:CV, :])
                dstage = d_pool.tile([CV + 1, CHW], F32, tag="dstage")
                nc.vector.tensor_copy(dstage[CV:CV + 1, :], o_ps[CV:CV + 1, :])
                k = ch // DBLK
                nc.gpsimd.dma_start(dm_tiles[k][ch % DBLK:ch % DBLK + 1, :],
                                    dstage[CV:CV + 1, :])
                del o_tiles[ch]
                if ch % DBLK == DBLK - 1:
                    # fold this block's denominators (outputs fire separately,
                    # one pair per two chunks, to avoid PSUM slot bursts)
                    lo = k * DBLK
                    fold_denominator(dm_tiles[k], lo, min(lo + DBLK, NCH))
                if ch >= 3:
                    phase6_pair(ch - 3, 1)

        # steady state per exp slot: PE does S(u+2) and O(u-1); ScalarE only exp
        s_group(0)
        s_group(1)
        for u in range(NU):
            if u + 2 < NU:
                s_group(u + 2)
            exp_group(u)
            if u >= 1:
                o_group(u - 1)
        o_group(NU - 1)

        # final fold (last partial block) + remaining outputs
        fold_denominator(dm_tiles[-1], (ndb - 1) * DBLK, NCH)
        for c in range(NCH - 3, NCH):
            phase6_pair(c, 1)

    nc.compile()
    return nc


_NC_CACHE = None


def _get_program():
    global _NC_CACHE
    if _NC_CACHE is None:
        _NC_CACHE = build_program()
    return _NC_CACHE


def kernel(**inputs) -> np.ndarray:
    from concourse.bass_utils import run_bass_kernel_spmd

    x = np.ascontiguousarray(np.asarray(inputs["x"], dtype=np.float32))
    B = x.shape[0]
    w_q = np.ascontiguousarray(np.asarray(inputs["w_q"], dtype=np.float32))
    w_k = np.ascontiguousarray(np.asarray(inputs["w_k"], dtype=np.float32))
    w_v = np.ascontiguousarray(np.asarray(inputs["w_v"], dtype=np.float32))
    w_o = np.ascontiguousarray(np.asarray(inputs["w_o"], dtype=np.float32))
    gamma = np.asarray(inputs["gamma"], dtype=np.float32).reshape(1, 1)

    nc = _get_program()
    in_maps = [
        {
            "x": x[b].reshape(C, NQ),
            "w_q": w_q, "w_k": w_k, "w_v": w_v, "w_o": w_o,
            "gamma": gamma,
        }
        for b in range(B)
    ]
    res = run_bass_kernel_spmd(nc, in_maps, core_ids=list(range(B)))
    out = np.stack([res.results[b]["out"].reshape(C, LL, LL, LL)
                    for b in range(B)])
    return out.astype(np.float32)


if __name__ == "__main__":
    nc = build_program()
    print("program built OK")


# revision 83
# speedup vs baseline: 1.0284x; 1.0284x over previous
"""Trainium2 Bass kernel for nn_Attention3d (3D attention with maxpooled K/V).

Reference computation per sample b:
    xf = x[b].reshape(C, Nq)                    C=128, Nq=24^3=13824
    q  = w_q @ xf                               [16, Nq]
    k  = maxpool2(w_k conv x)   -> [16, Nk]     Nk=12^3=1728
    v  = maxpool2(w_v conv x)   -> [64, Nk]
    attn = softmax_over_k(k^T q)                [Nk, Nq]
    o  = v @ attn                               [64, Nq]
    out = gamma * (w_o @ o) + xf

Sharding: data-parallel over batch B=8 -> 8 NeuronCores, one sample each.

Kernel structure (per core):
  1. conv phase: QKV 1x1 convs as matmuls (float32r, full rate)
  2. maxpool on DVE (3 in-place strided max stages)
  3. K replicated to 3 partition bases (row-packed S matmuls), V transposed
     via TensorE into V_T tiles with an appended ones-column (so the O matmul
     also produces the softmax denominator row for free)
  4. main loop over 27 query chunks of 512:
       S = K^T Q (bf16, 3-way row-tiled), exp on ScalarE (PSUM->SBUF, bf16),
       O = V_aug^T E (bf16 accumulation over 14 key tiles)
  5. denominator: reciprocal * gamma
  6. output loop: Y = w_o @ O (f32r), scale by broadcast 1/D, add residual, DMA out
"""

import numpy as np
from contextlib import ExitStack

import concourse.bacc as bacc
import concourse.bass as bass
import concourse.tile as tile
from concourse import mybir
from concourse.masks import make_identity

F32 = mybir.dt.float32
F32R = mybir.dt.float32r
BF16 = mybir.dt.bfloat16

C = 128
CA = 16
CV = 64
LL = 24
NQ = LL * LL * LL          # 13824
NKR = (LL // 2) ** 3       # 1728 real keys
NKT = 14                   # key tiles of 128 (last padded 64)
NKP = NKT * 128            # 1792
CHW = 512                  # query chunk width
NCH = NQ // CHW            # 27


def r32(ap):
    return ap.bitcast(F32R)


def build_program():
    nc = bacc.Bacc("TRN2", target_bir_lowering=False, debug=False, num_devices=8)

    x_d = nc.dram_tensor("x", [C, NQ], F32, kind="ExternalInput")
    wq_d = nc.dram_tensor("w_q", [CA, C], F32, kind="ExternalInput")
    wk_d = nc.dram_tensor("w_k", [CA, C], F32, kind="ExternalInput")
    wv_d = nc.dram_tensor("w_v", [CV, C], F32, kind="ExternalInput")
    wo_d = nc.dram_tensor("w_o", [C, CV], F32, kind="ExternalInput")
    g_d = nc.dram_tensor("gamma", [1, 1], F32, kind="ExternalInput")
    out_d = nc.dram_tensor("out", [C, NQ], F32, kind="ExternalOutput")
    dscr_d = nc.dram_tensor("dscr", [NCH, CHW], F32, kind="Internal")

    with tile.TileContext(nc) as tc, ExitStack() as ctx:
        singles = ctx.enter_context(tc.tile_pool(name="singles", bufs=1))
        big = ctx.enter_context(tc.tile_pool(name="big", bufs=1))
        e_pool = ctx.enter_context(tc.tile_pool(name="epool", bufs=5))
        r_pool = ctx.enter_context(tc.tile_pool(name="rpool", bufs=3))
        out_pool = ctx.enter_context(tc.tile_pool(name="outpool", bufs=3))

        # ---------------- constants / weights ----------------
        # lhsT for K/V conv: columns 0:64 = w_v^T (V on psum rows 0:64, base-0
        # for the PE transpose), columns 64:80 = w_k^T (K on rows 64:80)
        lhsT_kv = singles.tile([C, 80], F32)
        nc.scalar.dma_start(lhsT_kv[:, 0:64], wv_d.ap().transpose([1, 0]))
        nc.scalar.dma_start(lhsT_kv[:, 64:80], wk_d.ap().transpose([1, 0]))
        # lhsT for Q conv: w_q^T replicated at columns 0/32/64 so the conv
        # emits Q at partition bases 0/32/64 directly (for row-tiled S matmuls)
        lhsT_q = singles.tile([C, 80], F32)
        nc.vector.memset(lhsT_q[:, :], 0.0)
        for base, eng in ((0, nc.scalar), (32, nc.scalar), (64, nc.scalar)):
            eng.dma_start(lhsT_q[:, base:base + CA],
                          wq_d.ap().transpose([1, 0]))
        # w_o^T as [64, 128] bf16 (lhsT of the Y matmul)
        w_oT_f = singles.tile([CV, C], F32)
        nc.scalar.dma_start(w_oT_f[:, :], wo_d.ap().transpose([1, 0]))
        w_oT = singles.tile([CV, C], BF16)
        nc.vector.tensor_copy(w_oT[:, :], w_oT_f[:, :])
        # gamma broadcast down a column for the per-partition scalar multiply
        gamma_col = singles.tile([C, 1], F32)
        nc.scalar.dma_start(gamma_col[:, :], g_d.ap().to_broadcast((C, 1)))
        # identity for PE transpose (bf16 to match kv_sb); built later, after
        # the conv loop, so its gpsimd ops don't delay the conv-phase casts
        ident = singles.tile([C, C], BF16)

        # ---------------- big SBUF buffers ----------------
        x_pool = ctx.enter_context(tc.tile_pool(name="xstage", bufs=3))
        q_rep = big.tile([96, NQ], BF16)          # Q at partition bases 0/32/64
        o_buf = big.tile([CV, NQ], BF16)          # O rows 0:64 (bf16 for Y matmul)
        d_pool = ctx.enter_context(tc.tile_pool(name="dpool", bufs=3))
        k_rep = singles.tile([96, NKP], BF16)     # pooled K at bases 0/32/64, padded
        v_taug = singles.tile([C, NKT * (CV + 1)], BF16)  # V^T tiles + ones column
        # denominator blocks of chunks (base-0 tiles, folded incrementally)
        DBLK = 2
        ndb = (NCH + DBLK - 1) // DBLK
        dm_tiles = [singles.tile([min(DBLK, NCH - k * DBLK), CHW], F32,
                                 name=f"dm{k}") for k in range(ndb)]

        with tc.tile_pool(name="kvbuf", bufs=1) as kv_pool:
            kv_sb = kv_pool.tile([80, NQ], BF16)  # V/K conv out; pooled in place

            # ---------------- phase 1: conv (f32r = full-rate PE) ----------
            lhsT_kv_r = singles.tile([C, 80], F32R)
            nc.vector.tensor_copy(lhsT_kv_r[:, :], lhsT_kv[:, :])
            lhsT_q_r = singles.tile([C, 80], F32R)
            nc.vector.tensor_copy(lhsT_q_r[:, :], lhsT_q[:, :])
            with tc.tile_pool(name="convps", bufs=4, space="PSUM") as conv_ps:
                BLK = 2 * CHW  # 1024-wide input DMA blocks (fewer DMA issues)
                kvt = kv_sb[:, :]

                def kv_strided(off, dims):
                    return bass.AP(
                        tensor=kvt.tensor, offset=kvt.offset + off,
                        ap=[list(kvt.ap[0])] + [[s, c] for s, c in dims])

                for bi, blk in enumerate(range(0, NQ, BLK)):
                    bw = min(BLK, NQ - blk)
                    bsl = slice(blk, blk + bw)
                    xs = x_pool.tile([C, BLK], F32, tag="xs")
                    dma_eng = (nc.sync, nc.gpsimd)[bi % 2]
                    dma_eng.dma_start(xs[:, 0:bw], x_d.ap()[:, bsl])
                    xr = x_pool.tile([C, BLK], F32R, tag="xr")
                    nc.gpsimd.tensor_copy(xr[:, 0:bw], xs[:, 0:bw])
                    # 2-bank PSUM tiles; one matmul per bank-aligned half,
                    # then a single wide copy out (fewer ACT instructions)
                    cps = conv_ps.tile([80, 2 * CHW], F32, tag="cps")
                    qps = conv_ps.tile([80, 2 * CHW], F32, tag="cps")
                    for q0 in range(0, bw, CHW):
                        nc.tensor.matmul(cps[:, q0:q0 + CHW], lhsT_kv_r[:, :],
                                         xr[:, q0:q0 + CHW],
                                         start=True, stop=True)
                        nc.tensor.matmul(qps[:, q0:q0 + CHW], lhsT_q_r[:, :],
                                         xr[:, q0:q0 + CHW],
                                         start=True, stop=True)
                    # KV copy on ScalarE (idle until first exp);
                    # alternate Q copies between DVE and ScalarE
                    nc.scalar.copy(kv_sb[:, bsl], cps[:, 0:bw])
                    if bi % 2 == 0:
                        nc.vector.tensor_copy(q_rep[0:80, bsl], qps[:, 0:bw])
                    else:
                        nc.scalar.copy(q_rep[0:80, bsl], qps[:, 0:bw])
                    # maxpool stage 1 (w-pairs) for this block, in place:
                    # reads kv[blk : blk+bw], writes kv[blk/2 : blk/2+bw/2]
                    nc.vector.tensor_max(
                        kv_strided(blk // 2, [(1, bw // 2)]),
                        kv_strided(blk, [(2, bw // 2)]),
                        kv_strided(blk + 1, [(2, bw // 2)]))


            # ------------- phase 2: maxpool stages 2+3 (in place) -----------
            # kv_sb rows: 0:64 V, 64:80 K. After stage 1: idx = l*288+h*12+w2.
            # stage 2: pairs along h -> [80, 24*12*12]
            a2 = kv_strided(0, [(288, 24), (24, 12), (1, 12)])
            b2 = kv_strided(12, [(288, 24), (24, 12), (1, 12)])
            o2 = kv_strided(0, [(144, 24), (12, 12), (1, 12)])
            nc.vector.tensor_max(o2, a2, b2)
            # stage 3: pairs along l -> [80, 12^3]: pooled K/V in kv_sb[:, 0:1728]
            a3 = kv_strided(0, [(288, 12), (12, 12), (1, 12)])
            b3 = kv_strided(144, [(288, 12), (12, 12), (1, 12)])
            o3 = kv_strided(0, [(144, 12), (12, 12), (1, 12)])
            nc.vector.tensor_max(o3, a3, b3)

            # ---------- phase 3: K replication + V transpose ----------
            make_identity(nc, ident[:, :])
            nc.vector.memset(k_rep[:, :], 0.0)
            nc.vector.tensor_copy(k_rep[64:64 + CA, 0:NKR], kv_sb[64:80, 0:NKR])
            nc.sync.dma_start(k_rep[0:CA, :], k_rep[64:64 + CA, :])
            nc.scalar.dma_start(k_rep[32:32 + CA, :], k_rep[64:64 + CA, :])

            nc.vector.memset(v_taug[:, :], 0.0)
            with tc.tile_pool(name="tpps", bufs=2, space="PSUM") as tp_pool:
                for t in range(NKT):
                    cols = 128 if t < NKT - 1 else NKR - 128 * (NKT - 1)  # 64 last
                    tp = tp_pool.tile([C, CV], BF16, tag="tp")
                    nc.tensor.transpose(tp[0:cols, :],
                                        kv_sb[0:64, t * 128:t * 128 + cols],
                                        ident[0:64, 0:64])
                    nc.scalar.copy(v_taug[0:cols, t * 65:t * 65 + CV],
                                   tp[0:cols, :])
                    nc.gpsimd.memset(v_taug[0:cols, t * 65 + CV:t * 65 + CV + 1],
                                     1.0)

        # ---------------- phases 4-6: attention + output, one pipeline ------
        # Flat stream of (chunk, key-tile-group) units, software-pipelined so
        # ScalarE (exp, the bottleneck) runs back-to-back.  The output phase
        # for chunk c is interleaved once chunk c's denominator is ready
        # (denominators are folded in two halves).
        sps_pool = ctx.enter_context(tc.tile_pool(name="sps", bufs=2, space="PSUM"))
        ps_small = ctx.enter_context(tc.tile_pool(name="pssm", bufs=2, space="PSUM"))
        GROUPS = [(0, 3), (3, 3), (6, 3), (9, 3), (12, 2)]  # (tile0, ntiles)
        NG = len(GROUPS)
        HALF = 14  # denominator fold boundary (chunks 0:HALF, HALF:NCH)

        units = [(ch, g) for ch in range(NCH) for g in range(NG)]
        last_dstage = [None]
        NU = len(units)
        s_tiles = {}
        e_tiles = {}
        o_tiles = {}

        def s_group(u):
            ch, g = units[u]
            g0, gn = GROUPS[g]
            sl = bass.ts(ch, CHW)
            s_ps = sps_pool.tile([C, 3 * CHW], F32, tag="sps")
            s_tiles[u] = s_ps
            for t in range(g0, g0 + gn):
                j = t % 3
                nc.tensor.matmul(
                    s_ps[:, (t - g0) * CHW:(t - g0 + 1) * CHW],
                    k_rep[32 * j:32 * j + CA, t * 128:(t + 1) * 128],
                    q_rep[32 * j:32 * j + CA, sl],
                    start=True, stop=True,
                    tile_position=(32 * j, 0),
                )

        def exp_group(u):
            ch, g = units[u]
            g0, gn = GROUPS[g]
            et = e_pool.tile([C, 3 * CHW], BF16)
            nc.scalar.activation(et[:, 0:gn * CHW],
                                 s_tiles[u][:, 0:gn * CHW],
                                 mybir.ActivationFunctionType.Exp)
            e_tiles[u] = et
            del s_tiles[u]

        def fold_denominator(dm, lo, hi):
            n = hi - lo
            nc.vector.reciprocal(dm[0:n, :], dm[0:n, :])
            nc.vector.tensor_scalar_mul(dm[0:n, :], dm[0:n, :],
                                        gamma_col[0:n, :])
            nc.sync.dma_start(dscr_d.ap()[lo:hi, :], dm[0:n, :])

        def phase6_pair(c, ncc):
            """Output chunks c .. c+ncc-1 (ncc in {1,2}); paired DMAs."""
            w = ncc * CHW
            lo, hi = c * CHW, c * CHW + w
            r_sb = r_pool.tile([C, 2 * CHW], F32, name="r_sb")
            nc.gpsimd.dma_start(
                r_sb[:, 0:w],
                bass.AP(tensor=dscr_d.ap().tensor, offset=c * CHW,
                        ap=[[0, C], [1, w]]))
            xs6 = x_pool.tile([C, 2 * CHW], F32, tag="xs", name="xs6")
            nc.gpsimd.dma_start(xs6[:, 0:w], x_d.ap()[:, lo:hi])
            ot = out_pool.tile([C, 2 * CHW], F32, name="ot")
            for i in range(ncc):
                y_ps = ps_small.tile([C, CHW], F32, tag="ps", name="y_ps")
                nc.tensor.matmul(y_ps[:, :], w_oT[:, :],
                                 o_buf[0:CV, bass.ts(c + i, CHW)],
                                 start=True, stop=True)
                nc.vector.tensor_tensor(ot[:, bass.ts(i, CHW)], y_ps[:, :],
                                        r_sb[:, bass.ts(i, CHW)],
                                        mybir.AluOpType.mult)
            nc.vector.tensor_tensor(ot[:, 0:w], ot[:, 0:w], xs6[:, 0:w],
                                    mybir.AluOpType.add)
            nc.gpsimd.dma_start(out_d.ap()[:, lo:hi], ot[:, 0:w])

        def o_group(u):
            ch, g = units[u]
            g0, gn = GROUPS[g]
            sl = bass.ts(ch, CHW)
            if g == 0:
                o_tiles[ch] = ps_small.tile([CV + 1, CHW], F32, tag="ps",
                                            name="o_ps")
            o_ps = o_tiles[ch]
            et = e_tiles[u]
            for t in range(g0, g0 + gn):
                nc.tensor.matmul(
                    o_ps[:, :],
                    v_taug[:, t * 65:(t + 1) * 65],
                    et[:, (t - g0) * CHW:(t - g0 + 1) * CHW],
                    start=(t == 0), stop=(t == NKT - 1),
                )
            del e_tiles[u]
            if g == NG - 1:
                # chunk complete: export O and its denominator row
                nc.vector.tensor_copy(o_buf[:, sl], o_ps[0:CV, :])
                dstage = d_pool.tile([CV + 1, CHW], F32, tag="dstage")
                nc.vector.tensor_copy(dstage[CV:CV + 1, :], o_ps[CV:CV + 1, :])
                k = ch // DBLK
                nc.gpsimd.dma_start(dm_tiles[k][ch % DBLK:ch % DBLK + 1, :],
                                    dstage[CV:CV + 1, :])
                if ch % DBLK == DBLK - 1 or ch == NCH - 1:
                    # fold this block's denominators (outputs fire
                    # separately, one per chunk, to avoid PSUM bursts)
                    lo = k * DBLK
                    fold_denominator(dm_tiles[k], lo, min(lo + DBLK, NCH))
                del o_tiles[ch]
                if ch >= 2:
                    phase6_pair(ch - 2, 1)

        # steady state per exp slot: PE does S(u+2) and O(u-1); ScalarE only exp
        s_group(0)
        s_group(1)
        for u in range(NU):
            if u + 2 < NU:
                s_group(u + 2)
            exp_group(u)
            if u >= 1:
                o_group(u - 1)
        o_group(NU - 1)

        # tail: the final fold happens at chunk 26 completion (DBLK boundary);
        # emit the last two chunk outputs
        phase6_pair(NCH - 2, 1)
        phase6_pair(NCH - 1, 1)

    nc.compile()
    return nc


_NC_CACHE = None


def _get_program():
    global _NC_CACHE
    if _NC_CACHE is None:
        _NC_CACHE = build_program()
    return _NC_CACHE


def kernel(**inputs) -> np.ndarray:
    from concourse.bass_utils import run_bass_kernel_spmd

    x = np.ascontiguousarray(np.asarray(inputs["x"], dtype=np.float32))
    B = x.shape[0]
    w_q = np.ascontiguousarray(np.asarray(inputs["w_q"], dtype=np.float32))
    w_k = np.ascontiguousarray(np.asarray(inputs["w_k"], dtype=np.float32))
    w_v = np.ascontiguousarray(np.asarray(inputs["w_v"], dtype=np.float32))
    w_o = np.ascontiguousarray(np.asarray(inputs["w_o"], dtype=np.float32))
    gamma = np.asarray(inputs["gamma"], dtype=np.float32).reshape(1, 1)

    nc = _get_program()
    in_maps = [
        {
            "x": x[b].reshape(C, NQ),
            "w_q": w_q, "w_k": w_k, "w_v": w_v, "w_o": w_o,
            "gamma": gamma,
        }
        for b in range(B)
    ]
    res = run_bass_kernel_spmd(nc, in_maps, core_ids=list(range(B)))
    out = np.stack([res.results[b]["out"].reshape(C, LL, LL, LL)
                    for b in range(B)])
    return out.astype(np.float32)


if __name__ == "__main__":
    nc = build_program()
    print("program built OK")


# revision 87
# speedup vs baseline: 1.0485x; 1.0195x over previous
"""Trainium2 Bass kernel for nn_Attention3d (3D attention with maxpooled K/V).

Reference computation per sample b:
    xf = x[b].reshape(C, Nq)                    C=128, Nq=24^3=13824
    q  = w_q @ xf                               [16, Nq]
    k  = maxpool2(w_k conv x)   -> [16, Nk]     Nk=12^3=1728
    v  = maxpool2(w_v conv x)   -> [64, Nk]
    attn = softmax_over_k(k^T q)                [Nk, Nq]
    o  = v @ attn                               [64, Nq]
    out = gamma * (w_o @ o) + xf

Sharding: data-parallel over batch B=8 -> 8 NeuronCores, one sample each.

Kernel structure (per core):
  1. conv phase: QKV 1x1 convs as matmuls (float32r, full rate)
  2. maxpool on DVE (3 in-place strided max stages)
  3. K replicated to 3 partition bases (row-packed S matmuls), V transposed
     via TensorE into V_T tiles with an appended ones-column (so the O matmul
     also produces the softmax denominator row for free)
  4. main loop over 27 query chunks of 512:
       S = K^T Q (bf16, 3-way row-tiled), exp on ScalarE (PSUM->SBUF, bf16),
       O = V_aug^T E (bf16 accumulation over 14 key tiles)
  5. denominator: reciprocal * gamma
  6. output loop: Y = w_o @ O (f32r), scale by broadcast 1/D, add residual, DMA out
"""

import numpy as np
from contextlib import ExitStack

import concourse.bacc as bacc
import concourse.bass as bass
import concourse.tile as tile
from concourse import mybir
from concourse.masks import make_identity

F32 = mybir.dt.float32
F32R = mybir.dt.float32r
BF16 = mybir.dt.bfloat16

C = 128
CA = 16
CV = 64
LL = 24
NQ = LL * LL * LL          # 13824
NKR = (LL // 2) ** 3       # 1728 real keys
NKT = 14                   # key tiles of 128 (last padded 64)
NKP = NKT * 128            # 1792
CHW = 512                  # query chunk width
NCH = NQ // CHW            # 27


def r32(ap):
    return ap.bitcast(F32R)


def build_program():
    nc = bacc.Bacc("TRN2", target_bir_lowering=False, debug=False, num_devices=8)

    x_d = nc.dram_tensor("x", [C, NQ], F32, kind="ExternalInput")
    wq_d = nc.dram_tensor("w_q", [CA, C], F32, kind="ExternalInput")
    wk_d = nc.dram_tensor("w_k", [CA, C], F32, kind="ExternalInput")
    wv_d = nc.dram_tensor("w_v", [CV, C], F32, kind="ExternalInput")
    wo_d = nc.dram_tensor("w_o", [C, CV], F32, kind="ExternalInput")
    g_d = nc.dram_tensor("gamma", [1, 1], F32, kind="ExternalInput")
    out_d = nc.dram_tensor("out", [C, NQ], F32, kind="ExternalOutput")
    dscr_d = nc.dram_tensor("dscr", [NCH, CHW], F32, kind="Internal")

    with tile.TileContext(nc) as tc, ExitStack() as ctx:
        singles = ctx.enter_context(tc.tile_pool(name="singles", bufs=1))
        big = ctx.enter_context(tc.tile_pool(name="big", bufs=1))
        e_pool = ctx.enter_context(tc.tile_pool(name="epool", bufs=5))
        r_pool = ctx.enter_context(tc.tile_pool(name="rpool", bufs=3))
        out_pool = ctx.enter_context(tc.tile_pool(name="outpool", bufs=3))

        # ---------------- constants / weights ----------------
        # lhsT for K/V conv: columns 0:64 = w_v^T (V on psum rows 0:64, base-0
        # for the PE transpose), columns 64:80 = w_k^T (K on rows 64:80)
        lhsT_kv = singles.tile([C, 80], F32)
        nc.scalar.dma_start(lhsT_kv[:, 0:64], wv_d.ap().transpose([1, 0]))
        nc.scalar.dma_start(lhsT_kv[:, 64:80], wk_d.ap().transpose([1, 0]))
        # lhsT for Q conv: w_q^T replicated at columns 0/32/64 so the conv
        # emits Q at partition bases 0/32/64 directly (for row-tiled S matmuls)
        lhsT_q = singles.tile([C, 80], F32)
        nc.vector.memset(lhsT_q[:, :], 0.0)
        for base, eng in ((0, nc.scalar), (32, nc.scalar), (64, nc.scalar)):
            eng.dma_start(lhsT_q[:, base:base + CA],
                          wq_d.ap().transpose([1, 0]))
        # w_o^T as [64, 128] bf16 (lhsT of the Y matmul)
        w_oT_f = singles.tile([CV, C], F32)
        nc.scalar.dma_start(w_oT_f[:, :], wo_d.ap().transpose([1, 0]))
        w_oT = singles.tile([CV, C], BF16)
        nc.vector.tensor_copy(w_oT[:, :], w_oT_f[:, :])
        # gamma broadcast down a column for the per-partition scalar multiply
        gamma_col = singles.tile([C, 1], F32)
        nc.scalar.dma_start(gamma_col[:, :], g_d.ap().to_broadcast((C, 1)))
        # identity for PE transpose (bf16 to match kv_sb); built later, after
        # the conv loop, so its gpsimd ops don't delay the conv-phase casts
        ident = singles.tile([C, C], BF16)

        # ---------------- big SBUF buffers ----------------
        x_pool = ctx.enter_context(tc.tile_pool(name="xstage", bufs=3))
        q_rep = big.tile([96, NQ], BF16)          # Q at partition bases 0/32/64
        o_buf = big.tile([CV, NQ], BF16)          # O rows 0:64 (bf16 for Y matmul)
        d_pool = ctx.enter_context(tc.tile_pool(name="dpool", bufs=3))
        k_rep = singles.tile([96, NKP], BF16)     # pooled K at bases 0/32/64, padded
        v_taug = singles.tile([C, NKT * (CV + 1)], BF16)  # V^T tiles + ones column
        # denominator blocks of chunks (base-0 tiles, folded incrementally)
        DBLK = 2
        ndb = (NCH + DBLK - 1) // DBLK
        dm_tiles = [singles.tile([min(DBLK, NCH - k * DBLK), CHW], F32,
                                 name=f"dm{k}") for k in range(ndb)]

        with tc.tile_pool(name="kvbuf", bufs=1) as kv_pool:
            kv_sb = kv_pool.tile([80, NQ], BF16)  # V/K conv out; pooled in place

            # ---------------- phase 1: conv (f32r = full-rate PE) ----------
            lhsT_kv_r = singles.tile([C, 80], F32R)
            nc.vector.tensor_copy(lhsT_kv_r[:, :], lhsT_kv[:, :])
            lhsT_q_r = singles.tile([C, 80], F32R)
            nc.vector.tensor_copy(lhsT_q_r[:, :], lhsT_q[:, :])
            with tc.tile_pool(name="convps", bufs=4, space="PSUM") as conv_ps:
                BLK = 2 * CHW  # 1024-wide input DMA blocks (fewer DMA issues)
                kvt = kv_sb[:, :]

                def kv_strided(off, dims):
                    return bass.AP(
                        tensor=kvt.tensor, offset=kvt.offset + off,
                        ap=[list(kvt.ap[0])] + [[s, c] for s, c in dims])

                for bi, blk in enumerate(range(0, NQ, BLK)):
                    bw = min(BLK, NQ - blk)
                    bsl = slice(blk, blk + bw)
                    xs = x_pool.tile([C, BLK], F32, tag="xs")
                    dma_eng = (nc.sync, nc.gpsimd)[bi % 2]
                    dma_eng.dma_start(xs[:, 0:bw], x_d.ap()[:, bsl])
                    xr = x_pool.tile([C, BLK], F32R, tag="xr")
                    nc.gpsimd.tensor_copy(xr[:, 0:bw], xs[:, 0:bw])
                    # 2-bank PSUM tiles; one matmul per bank-aligned half,
                    # then a single wide copy out (fewer ACT instructions)
                    cps = conv_ps.tile([80, 2 * CHW], F32, tag="cps")
                    qps = conv_ps.tile([80, 2 * CHW], F32, tag="cps")
                    for q0 in range(0, bw, CHW):
                        nc.tensor.matmul(cps[:, q0:q0 + CHW], lhsT_kv_r[:, :],
                                         xr[:, q0:q0 + CHW],
                                         start=True, stop=True)
                        nc.tensor.matmul(qps[:, q0:q0 + CHW], lhsT_q_r[:, :],
                                         xr[:, q0:q0 + CHW],
                                         start=True, stop=True)
                    # KV copy on ScalarE (idle until first exp);
                    # alternate Q copies between DVE and ScalarE
                    nc.scalar.copy(kv_sb[:, bsl], cps[:, 0:bw])
                    if bi % 2 == 0:
                        nc.vector.tensor_copy(q_rep[0:80, bsl], qps[:, 0:bw])
                    else:
                        nc.scalar.copy(q_rep[0:80, bsl], qps[:, 0:bw])
                    # maxpool stage 1 (w-pairs) for this block, in place:
                    # reads kv[blk : blk+bw], writes kv[blk/2 : blk/2+bw/2]
                    nc.vector.tensor_max(
                        kv_strided(blk // 2, [(1, bw // 2)]),
                        kv_strided(blk, [(2, bw // 2)]),
                        kv_strided(blk + 1, [(2, bw // 2)]))


            # ------------- phase 2: maxpool stages 2+3 (in place) -----------
            # kv_sb rows: 0:64 V, 64:80 K. After stage 1: idx = l*288+h*12+w2.
            # stage 2: pairs along h -> [80, 24*12*12]
            nc.vector.tensor_max(
                kv_strided(0, [(144, 24), (12, 12), (1, 12)]),
                kv_strided(0, [(288, 24), (24, 12), (1, 12)]),
                kv_strided(12, [(288, 24), (24, 12), (1, 12)]))
            # stage 3: pairs along l -> [80, 12^3]: pooled K/V in [:, 0:1728]
            nc.vector.tensor_max(
                kv_strided(0, [(144, 12), (12, 12), (1, 12)]),
                kv_strided(0, [(288, 12), (12, 12), (1, 12)]),
                kv_strided(144, [(288, 12), (12, 12), (1, 12)]))

            # ---------- phase 3: K replication + V transpose ----------
            nc.vector.memset(k_rep[:, :], 0.0)
            nc.vector.tensor_copy(k_rep[64:64 + CA, 0:NKR], kv_sb[64:80, 0:NKR])
            nc.sync.dma_start(k_rep[0:CA, :], k_rep[64:64 + CA, :])
            nc.scalar.dma_start(k_rep[32:32 + CA, :], k_rep[64:64 + CA, :])

            make_identity(nc, ident[:, :])
            nc.vector.memset(v_taug[:, :], 0.0)
            with tc.tile_pool(name="tpps", bufs=2, space="PSUM") as tp_pool:
                for t in range(NKT):
                    cols = 128 if t < NKT - 1 else NKR - 128 * (NKT - 1)  # 64 last
                    tp = tp_pool.tile([C, CV], BF16, tag="tp")
                    nc.tensor.transpose(tp[0:cols, :],
                                        kv_sb[0:64, t * 128:t * 128 + cols],
                                        ident[0:64, 0:64])
                    nc.scalar.copy(v_taug[0:cols, t * 65:t * 65 + CV],
                                   tp[0:cols, :])
                    nc.gpsimd.memset(v_taug[0:cols, t * 65 + CV:t * 65 + CV + 1],
                                     1.0)

        # ---------------- phases 4-6: attention + output, one pipeline ------
        # Flat stream of (chunk, key-tile-group) units, software-pipelined so
        # ScalarE (exp, the bottleneck) runs back-to-back.  The output phase
        # for chunk c is interleaved once chunk c's denominator is ready
        # (denominators are folded in two halves).
        sps_pool = ctx.enter_context(tc.tile_pool(name="sps", bufs=2, space="PSUM"))
        ps_small = ctx.enter_context(tc.tile_pool(name="pssm", bufs=2, space="PSUM"))
        GROUPS = [(0, 3), (3, 3), (6, 3), (9, 3), (12, 2)]  # (tile0, ntiles)
        NG = len(GROUPS)
        HALF = 14  # denominator fold boundary (chunks 0:HALF, HALF:NCH)

        units = [(ch, g) for ch in range(NCH) for g in range(NG)]
        last_dstage = [None]
        NU = len(units)
        s_tiles = {}
        e_tiles = {}
        o_tiles = {}

        def s_group(u):
            ch, g = units[u]
            g0, gn = GROUPS[g]
            sl = bass.ts(ch, CHW)
            s_ps = sps_pool.tile([C, 3 * CHW], F32, tag="sps")
            s_tiles[u] = s_ps
            for t in range(g0, g0 + gn):
                j = t % 3
                nc.tensor.matmul(
                    s_ps[:, (t - g0) * CHW:(t - g0 + 1) * CHW],
                    k_rep[32 * j:32 * j + CA, t * 128:(t + 1) * 128],
                    q_rep[32 * j:32 * j + CA, sl],
                    start=True, stop=True,
                    tile_position=(32 * j, 0),
                )

        def exp_group(u):
            ch, g = units[u]
            g0, gn = GROUPS[g]
            et = e_pool.tile([C, 3 * CHW], BF16)
            nc.scalar.activation(et[:, 0:gn * CHW],
                                 s_tiles[u][:, 0:gn * CHW],
                                 mybir.ActivationFunctionType.Exp)
            e_tiles[u] = et
            del s_tiles[u]

        def fold_denominator(dm, lo, hi):
            n = hi - lo
            nc.vector.reciprocal(dm[0:n, :], dm[0:n, :])
            nc.vector.tensor_scalar_mul(dm[0:n, :], dm[0:n, :],
                                        gamma_col[0:n, :])
            nc.sync.dma_start(dscr_d.ap()[lo:hi, :], dm[0:n, :])

        def phase6_pair(c, ncc):
            """Output chunks c .. c+ncc-1 (ncc in {1,2}); paired DMAs."""
            w = ncc * CHW
            lo, hi = c * CHW, c * CHW + w
            r_sb = r_pool.tile([C, 2 * CHW], F32, name="r_sb")
            nc.gpsimd.dma_start(
                r_sb[:, 0:w],
                bass.AP(tensor=dscr_d.ap().tensor, offset=c * CHW,
                        ap=[[0, C], [1, w]]))
            xs6 = x_pool.tile([C, 2 * CHW], F32, tag="xs", name="xs6")
            nc.gpsimd.dma_start(xs6[:, 0:w], x_d.ap()[:, lo:hi])
            ot = out_pool.tile([C, 2 * CHW], F32, name="ot")
            for i in range(ncc):
                y_ps = ps_small.tile([C, CHW], F32, tag="ps", name="y_ps")
                nc.tensor.matmul(y_ps[:, :], w_oT[:, :],
                                 o_buf[0:CV, bass.ts(c + i, CHW)],
                                 start=True, stop=True)
                nc.vector.tensor_tensor(ot[:, bass.ts(i, CHW)], y_ps[:, :],
                                        r_sb[:, bass.ts(i, CHW)],
                                        mybir.AluOpType.mult)
            nc.vector.tensor_tensor(ot[:, 0:w], ot[:, 0:w], xs6[:, 0:w],
                                    mybir.AluOpType.add)
            nc.gpsimd.dma_start(out_d.ap()[:, lo:hi], ot[:, 0:w])

        def o_group(u):
            ch, g = units[u]
            g0, gn = GROUPS[g]
            sl = bass.ts(ch, CHW)
            if g == 0:
                o_tiles[ch] = ps_small.tile([CV + 1, CHW], F32, tag="ps",
                                            name="o_ps")
            o_ps = o_tiles[ch]
            et = e_tiles[u]
            for t in range(g0, g0 + gn):
                nc.tensor.matmul(
                    o_ps[:, :],
                    v_taug[:, t * 65:(t + 1) * 65],
                    et[:, (t - g0) * CHW:(t - g0 + 1) * CHW],
                    start=(t == 0), stop=(t == NKT - 1),
                )
            del e_tiles[u]
            if g == NG - 1:
                # chunk complete: export O and its denominator row
                nc.vector.tensor_copy(o_buf[:, sl], o_ps[0:CV, :])
                dstage = d_pool.tile([CV + 1, CHW], F32, tag="dstage")
                nc.vector.tensor_copy(dstage[CV:CV + 1, :], o_ps[CV:CV + 1, :])
                if ch == NCH - 1:
                    # last chunk: fold in place on the staging row (partition
                    # 64 is a legal DVE base) -- skips the dm-tile hop
                    dl = dstage[CV:CV + 1, :]
                    nc.vector.reciprocal(dl, dl)
                    nc.vector.tensor_scalar_mul(dl, dl,
                                                gamma_col[CV:CV + 1, :])
                    nc.sync.dma_start(dscr_d.ap()[ch:ch + 1, :], dl)
                else:
                    k = ch // DBLK
                    nc.gpsimd.dma_start(
                        dm_tiles[k][ch % DBLK:ch % DBLK + 1, :],
                        dstage[CV:CV + 1, :])
                    if ch % DBLK == DBLK - 1:
                        # fold this block's denominators (outputs fire
                        # separately, one per chunk, to avoid PSUM bursts)
                        lo = k * DBLK
                        fold_denominator(dm_tiles[k], lo,
                                         min(lo + DBLK, NCH - 1))
                del o_tiles[ch]
                if ch >= 2:
                    phase6_pair(ch - 2, 1)

        # steady state per exp slot: PE does S(u+2) and O(u-1); ScalarE only exp
        s_group(0)
        s_group(1)
        for u in range(NU):
            if u + 2 < NU:
                s_group(u + 2)
            exp_group(u)
            if u >= 1:
                o_group(u - 1)
        o_group(NU - 1)

        # tail: the final fold happens at chunk 26 completion (DBLK boundary);
        # emit the last two chunk outputs
        phase6_pair(NCH - 2, 1)
        phase6_pair(NCH - 1, 1)

    nc.compile()
    return nc


_NC_CACHE = None


def _get_program():
    global _NC_CACHE
    if _NC_CACHE is None:
        _NC_CACHE = build_program()
    return _NC_CACHE


def kernel(**inputs) -> np.ndarray:
    from concourse.bass_utils import run_bass_kernel_spmd

    x = np.ascontiguousarray(np.asarray(inputs["x"], dtype=np.float32))
    B = x.shape[0]
    w_q = np.ascontiguousarray(np.asarray(inputs["w_q"], dtype=np.float32))
    w_k = np.ascontiguousarray(np.asarray(inputs["w_k"], dtype=np.float32))
    w_v = np.ascontiguousarray(np.asarray(inputs["w_v"], dtype=np.float32))
    w_o = np.ascontiguousarray(np.asarray(inputs["w_o"], dtype=np.float32))
    gamma = np.asarray(inputs["gamma"], dtype=np.float32).reshape(1, 1)

    nc = _get_program()
    in_maps = [
        {
            "x": x[b].reshape(C, NQ),
            "w_q": w_q, "w_k": w_k, "w_v": w_v, "w_o": w_o,
            "gamma": gamma,
        }
        for b in range(B)
    ]
    res = run_bass_kernel_spmd(nc, in_maps, core_ids=list(range(B)))
    out = np.stack([res.results[b]["out"].reshape(C, LL, LL, LL)
                    for b in range(B)])
    return out.astype(np.float32)


if __name__ == "__main__":
    nc = build_program()
    print("program built OK")


# revision 88
# speedup vs baseline: 1.1523x; 1.0990x over previous
"""Trainium2 Bass kernel for nn_Attention3d (3D attention with maxpooled K/V).

Reference computation per sample b:
    xf = x[b].reshape(C, Nq)                    C=128, Nq=24^3=13824
    q  = w_q @ xf                               [16, Nq]
    k  = maxpool2(w_k conv x)   -> [16, Nk]     Nk=12^3=1728
    v  = maxpool2(w_v conv x)   -> [64, Nk]
    attn = softmax_over_k(k^T q)                [Nk, Nq]
    o  = v @ attn                               [64, Nq]
    out = gamma * (w_o @ o) + xf

Sharding: data-parallel over batch B=8 -> 8 NeuronCores, one sample each.

Kernel structure (per core):
  1. conv phase: QKV 1x1 convs as matmuls (float32r = full-rate fp32; Q is
     emitted at partition bases 0/32/64 via replicated lhsT columns); maxpool
     stage 1 (w-pairs) folded into the conv loop, in place
  2. maxpool stages 2+3 on DVE (in-place strided max)
  3. K replicated to 3 partition bases (row-tiled S matmuls), V transposed
     via TensorE into V^T tiles with an appended ones-column (so the O matmul
     also produces the softmax denominator row for free)
  4. flat software-pipelined stream over (chunk, key-tile-group) units:
       S = K^T Q (bf16, 3-way row-tiled, two groups ahead), exp on ScalarE
       (back-to-back, the bottleneck), O = V_aug^T E one group behind
  5. denominators folded incrementally (reciprocal * gamma, blocks of 2
     chunks); outputs (Y = w_o @ O in bf16, broadcast 1/D, fp32 residual)
     fire one chunk per completion at lag 2, overlapped with the main loop
"""

import numpy as np
from contextlib import ExitStack

import concourse.bacc as bacc
import concourse.bass as bass
import concourse.tile as tile
from concourse import mybir
from concourse.masks import make_identity

F32 = mybir.dt.float32
F32R = mybir.dt.float32r
BF16 = mybir.dt.bfloat16

C = 128
CA = 16
CV = 64
LL = 24
NQ = LL * LL * LL          # 13824
NKR = (LL // 2) ** 3       # 1728 real keys
NKT = 14                   # key tiles of 128 (last padded 64)
NKP = NKT * 128            # 1792
CHW = 512                  # query chunk width
NCH = NQ // CHW            # 27


def r32(ap):
    return ap.bitcast(F32R)


def build_program():
    nc = bacc.Bacc("TRN2", target_bir_lowering=False, debug=False, num_devices=8)

    x_d = nc.dram_tensor("x", [C, NQ], F32, kind="ExternalInput")
    wq_d = nc.dram_tensor("w_q", [CA, C], F32, kind="ExternalInput")
    wk_d = nc.dram_tensor("w_k", [CA, C], F32, kind="ExternalInput")
    wv_d = nc.dram_tensor("w_v", [CV, C], F32, kind="ExternalInput")
    wo_d = nc.dram_tensor("w_o", [C, CV], F32, kind="ExternalInput")
    g_d = nc.dram_tensor("gamma", [1, 1], F32, kind="ExternalInput")
    out_d = nc.dram_tensor("out", [C, NQ], F32, kind="ExternalOutput")
    dscr_d = nc.dram_tensor("dscr", [NCH, CHW], F32, kind="Internal")

    with tile.TileContext(nc) as tc, ExitStack() as ctx:
        singles = ctx.enter_context(tc.tile_pool(name="singles", bufs=1))
        big = ctx.enter_context(tc.tile_pool(name="big", bufs=1))
        e_pool = ctx.enter_context(tc.tile_pool(name="epool", bufs=5))
        r_pool = ctx.enter_context(tc.tile_pool(name="rpool", bufs=3))
        out_pool = ctx.enter_context(tc.tile_pool(name="outpool", bufs=3))

        # ---------------- constants / weights ----------------
        # lhsT for K/V conv: columns 0:64 = w_v^T (V on psum rows 0:64, base-0
        # for the PE transpose), columns 64:80 = w_k^T (K on rows 64:80)
        lhsT_kv = singles.tile([C, 80], F32)
        nc.scalar.dma_start(lhsT_kv[:, 0:64], wv_d.ap().transpose([1, 0]))
        nc.scalar.dma_start(lhsT_kv[:, 64:80], wk_d.ap().transpose([1, 0]))
        # lhsT for Q conv: w_q^T replicated at columns 0/32/64 so the conv
        # emits Q at partition bases 0/32/64 directly (for row-tiled S matmuls)
        lhsT_q = singles.tile([C, 80], F32)
        nc.vector.memset(lhsT_q[:, :], 0.0)
        for base, eng in ((0, nc.scalar), (32, nc.scalar), (64, nc.scalar)):
            eng.dma_start(lhsT_q[:, base:base + CA],
                          wq_d.ap().transpose([1, 0]))
        # w_o^T as [64, 128] bf16 (lhsT of the Y matmul)
        w_oT_f = singles.tile([CV, C], F32)
        nc.scalar.dma_start(w_oT_f[:, :], wo_d.ap().transpose([1, 0]))
        w_oT = singles.tile([CV, C], BF16)
        nc.vector.tensor_copy(w_oT[:, :], w_oT_f[:, :])
        # gamma broadcast down a column for the per-partition scalar multiply
        gamma_col = singles.tile([C, 1], F32)
        nc.scalar.dma_start(gamma_col[:, :], g_d.ap().to_broadcast((C, 1)))
        # identity for PE transpose (bf16 to match kv_sb); built later, after
        # the conv loop, so its gpsimd ops don't delay the conv-phase casts
        ident = singles.tile([C, C], BF16)

        # ---------------- big SBUF buffers ----------------
        x_pool = ctx.enter_context(tc.tile_pool(name="xstage", bufs=3))
        q_rep = big.tile([96, NQ], BF16)          # Q at partition bases 0/32/64
        o_buf = big.tile([CV, NQ], BF16)          # O rows 0:64 (bf16 for Y matmul)
        d_pool = ctx.enter_context(tc.tile_pool(name="dpool", bufs=3))
        k_rep = singles.tile([96, NKP], BF16)     # pooled K at bases 0/32/64, padded
        v_taug = singles.tile([C, NKT * (CV + 1)], BF16)  # V^T tiles + ones column
        # denominator blocks of chunks (base-0 tiles, folded incrementally)
        DBLK = 2
        ndb = (NCH + DBLK - 1) // DBLK
        dm_tiles = [singles.tile([min(DBLK, NCH - k * DBLK), CHW], F32,
                                 name=f"dm{k}") for k in range(ndb)]

        with tc.tile_pool(name="kvbuf", bufs=1) as kv_pool:
            kv_sb = kv_pool.tile([80, NQ], BF16)  # V/K conv out; pooled in place

            # ---------------- phase 1: conv (f32r = full-rate PE) ----------
            lhsT_kv_r = singles.tile([C, 80], F32R)
            nc.vector.tensor_copy(lhsT_kv_r[:, :], lhsT_kv[:, :])
            lhsT_q_r = singles.tile([C, 80], F32R)
            nc.vector.tensor_copy(lhsT_q_r[:, :], lhsT_q[:, :])
            with tc.tile_pool(name="convps", bufs=4, space="PSUM") as conv_ps:
                BLK = 2 * CHW  # 1024-wide input DMA blocks (fewer DMA issues)
                kvt = kv_sb[:, :]

                def kv_strided(off, dims):
                    return bass.AP(
                        tensor=kvt.tensor, offset=kvt.offset + off,
                        ap=[list(kvt.ap[0])] + [[s, c] for s, c in dims])

                for bi, blk in enumerate(range(0, NQ, BLK)):
                    bw = min(BLK, NQ - blk)
                    bsl = slice(blk, blk + bw)
                    xs = x_pool.tile([C, BLK], F32, tag="xs")
                    dma_eng = (nc.sync, nc.gpsimd)[bi % 2]
                    dma_eng.dma_start(xs[:, 0:bw], x_d.ap()[:, bsl])
                    xr = x_pool.tile([C, BLK], F32R, tag="xr")
                    nc.gpsimd.tensor_copy(xr[:, 0:bw], xs[:, 0:bw])
                    # 2-bank PSUM tiles; one matmul per bank-aligned half,
                    # then a single wide copy out (fewer ACT instructions)
                    cps = conv_ps.tile([80, 2 * CHW], F32, tag="cps")
                    qps = conv_ps.tile([80, 2 * CHW], F32, tag="cps")
                    for q0 in range(0, bw, CHW):
                        nc.tensor.matmul(cps[:, q0:q0 + CHW], lhsT_kv_r[:, :],
                                         xr[:, q0:q0 + CHW],
                                         start=True, stop=True)
                        nc.tensor.matmul(qps[:, q0:q0 + CHW], lhsT_q_r[:, :],
                                         xr[:, q0:q0 + CHW],
                                         start=True, stop=True)
                    # KV copy on ScalarE (idle until first exp);
                    # alternate Q copies between DVE and ScalarE
                    nc.scalar.copy(kv_sb[:, bsl], cps[:, 0:bw])
                    if bi % 2 == 0:
                        nc.vector.tensor_copy(q_rep[0:80, bsl], qps[:, 0:bw])
                    else:
                        nc.scalar.copy(q_rep[0:80, bsl], qps[:, 0:bw])
                    # maxpool stage 1 (w-pairs) for this block, in place:
                    # reads kv[blk : blk+bw], writes kv[blk/2 : blk/2+bw/2]
                    nc.vector.tensor_max(
                        kv_strided(blk // 2, [(1, bw // 2)]),
                        kv_strided(blk, [(2, bw // 2)]),
                        kv_strided(blk + 1, [(2, bw // 2)]))


            # ------------- phase 2: maxpool stages 2+3 (in place) -----------
            # kv_sb rows: 0:64 V, 64:80 K. After stage 1: idx = l*288+h*12+w2.
            # stage 2: pairs along h -> [80, 24*12*12]
            nc.vector.tensor_max(
                kv_strided(0, [(144, 24), (12, 12), (1, 12)]),
                kv_strided(0, [(288, 24), (24, 12), (1, 12)]),
                kv_strided(12, [(288, 24), (24, 12), (1, 12)]))
            # stage 3: pairs along l -> [80, 12^3]: pooled K/V in [:, 0:1728]
            nc.vector.tensor_max(
                kv_strided(0, [(144, 12), (12, 12), (1, 12)]),
                kv_strided(0, [(288, 12), (12, 12), (1, 12)]),
                kv_strided(144, [(288, 12), (12, 12), (1, 12)]))

            # ---------- phase 3: K replication + V transpose ----------
            nc.vector.memset(k_rep[:, :], 0.0)
            nc.vector.tensor_copy(k_rep[64:64 + CA, 0:NKR], kv_sb[64:80, 0:NKR])
            nc.sync.dma_start(k_rep[0:CA, :], k_rep[64:64 + CA, :])
            nc.scalar.dma_start(k_rep[32:32 + CA, :], k_rep[64:64 + CA, :])

            make_identity(nc, ident[:, :])
            nc.vector.memset(v_taug[:, :], 0.0)
            with tc.tile_pool(name="tpps", bufs=2, space="PSUM") as tp_pool:
                for t in range(NKT):
                    cols = 128 if t < NKT - 1 else NKR - 128 * (NKT - 1)  # 64 last
                    tp = tp_pool.tile([C, CV], BF16, tag="tp")
                    nc.tensor.transpose(tp[0:cols, :],
                                        kv_sb[0:64, t * 128:t * 128 + cols],
                                        ident[0:64, 0:64])
                    nc.scalar.copy(v_taug[0:cols, t * 65:t * 65 + CV],
                                   tp[0:cols, :])
                    nc.gpsimd.memset(v_taug[0:cols, t * 65 + CV:t * 65 + CV + 1],
                                     1.0)

        # ---------------- phases 4-6: attention + output, one pipeline ------
        # Flat stream of (chunk, key-tile-group) units, software-pipelined so
        # ScalarE (exp, the bottleneck) runs back-to-back.  The output phase
        # for chunk c is interleaved once chunk c's denominator is ready
        # (denominators are folded in two halves).
        sps_pool = ctx.enter_context(tc.tile_pool(name="sps", bufs=2, space="PSUM"))
        ps_small = ctx.enter_context(tc.tile_pool(name="pssm", bufs=2, space="PSUM"))
        GROUPS = [(0, 3), (3, 3), (6, 3), (9, 3), (12, 2)]  # (tile0, ntiles)
        NG = len(GROUPS)
        HALF = 14  # denominator fold boundary (chunks 0:HALF, HALF:NCH)

        units = [(ch, g) for ch in range(NCH) for g in range(NG)]
        last_dstage = [None]
        NU = len(units)
        s_tiles = {}
        e_tiles = {}
        o_tiles = {}

        def s_group(u):
            ch, g = units[u]
            g0, gn = GROUPS[g]
            sl = bass.ts(ch, CHW)
            s_ps = sps_pool.tile([C, 3 * CHW], F32, tag="sps")
            s_tiles[u] = s_ps
            for t in range(g0, g0 + gn):
                j = t % 3
                nc.tensor.matmul(
                    s_ps[:, (t - g0) * CHW:(t - g0 + 1) * CHW],
                    k_rep[32 * j:32 * j + CA, t * 128:(t + 1) * 128],
                    q_rep[32 * j:32 * j + CA, sl],
                    start=True, stop=True,
                    tile_position=(32 * j, 0),
                )

        def exp_group(u):
            ch, g = units[u]
            g0, gn = GROUPS[g]
            et = e_pool.tile([C, 3 * CHW], BF16)
            nc.scalar.activation(et[:, 0:gn * CHW],
                                 s_tiles[u][:, 0:gn * CHW],
                                 mybir.ActivationFunctionType.Exp)
            e_tiles[u] = et
            del s_tiles[u]

        def fold_denominator(dm, lo, hi):
            n = hi - lo
            nc.vector.reciprocal(dm[0:n, :], dm[0:n, :])
            nc.vector.tensor_scalar_mul(dm[0:n, :], dm[0:n, :],
                                        gamma_col[0:n, :])
            nc.sync.dma_start(dscr_d.ap()[lo:hi, :], dm[0:n, :])

        def phase6_pair(c, ncc):
            """Output chunks c .. c+ncc-1 (ncc in {1,2}); paired DMAs."""
            w = ncc * CHW
            lo, hi = c * CHW, c * CHW + w
            r_sb = r_pool.tile([C, 2 * CHW], F32, name="r_sb")
            nc.gpsimd.dma_start(
                r_sb[:, 0:w],
                bass.AP(tensor=dscr_d.ap().tensor, offset=c * CHW,
                        ap=[[0, C], [1, w]]))
            xs6 = x_pool.tile([C, 2 * CHW], F32, tag="xs", name="xs6")
            nc.gpsimd.dma_start(xs6[:, 0:w], x_d.ap()[:, lo:hi])
            ot = out_pool.tile([C, 2 * CHW], F32, name="ot")
            for i in range(ncc):
                y_ps = ps_small.tile([C, CHW], F32, tag="ps", name="y_ps")
                nc.tensor.matmul(y_ps[:, :], w_oT[:, :],
                                 o_buf[0:CV, bass.ts(c + i, CHW)],
                                 start=True, stop=True)
                nc.vector.tensor_tensor(ot[:, bass.ts(i, CHW)], y_ps[:, :],
                                        r_sb[:, bass.ts(i, CHW)],
                                        mybir.AluOpType.mult)
            nc.vector.tensor_tensor(ot[:, 0:w], ot[:, 0:w], xs6[:, 0:w],
                                    mybir.AluOpType.add)
            nc.gpsimd.dma_start(out_d.ap()[:, lo:hi], ot[:, 0:w])

        def o_group(u):
            ch, g = units[u]
            g0, gn = GROUPS[g]
            sl = bass.ts(ch, CHW)
            if g == 0:
                o_tiles[ch] = ps_small.tile([CV + 1, CHW], F32, tag="ps",
                                            name="o_ps")
            o_ps = o_tiles[ch]
            et = e_tiles[u]
            for t in range(g0, g0 + gn):
                nc.tensor.matmul(
                    o_ps[:, :],
                    v_taug[:, t * 65:(t + 1) * 65],
                    et[:, (t - g0) * CHW:(t - g0 + 1) * CHW],
                    start=(t == 0), stop=(t == NKT - 1),
                )
            del e_tiles[u]
            if g == NG - 1:
                # chunk complete: export O and its denominator row
                nc.vector.tensor_copy(o_buf[:, sl], o_ps[0:CV, :])
                dstage = d_pool.tile([CV + 1, CHW], F32, tag="dstage")
                nc.vector.tensor_copy(dstage[CV:CV + 1, :], o_ps[CV:CV + 1, :])
                if ch == NCH - 1:
                    # last chunk: fold in place on the staging row (partition
                    # 64 is a legal DVE base) -- skips the dm-tile hop
                    dl = dstage[CV:CV + 1, :]
                    nc.vector.reciprocal(dl, dl)
                    nc.vector.tensor_scalar_mul(dl, dl,
                                                gamma_col[CV:CV + 1, :])
                    nc.sync.dma_start(dscr_d.ap()[ch:ch + 1, :], dl)
                else:
                    k = ch // DBLK
                    nc.gpsimd.dma_start(
                        dm_tiles[k][ch % DBLK:ch % DBLK + 1, :],
                        dstage[CV:CV + 1, :])
                    if ch % DBLK == DBLK - 1:
                        # fold this block's denominators (outputs fire
                        # separately, one per chunk, to avoid PSUM bursts)
                        lo = k * DBLK
                        fold_denominator(dm_tiles[k], lo,
                                         min(lo + DBLK, NCH - 1))
                del o_tiles[ch]
                if ch >= 2:
                    phase6_pair(ch - 2, 1)

        # steady state per exp slot: PE does S(u+2) and O(u-1); ScalarE only exp
        s_group(0)
        s_group(1)
        for u in range(NU):
            if u + 2 < NU:
                s_group(u + 2)
            exp_group(u)
            if u >= 1:
                o_group(u - 1)
        o_group(NU - 1)

        # tail: the final fold happens at chunk 26 completion (DBLK boundary);
        # emit the last two chunk outputs
        phase6_pair(NCH - 2, 1)
        phase6_pair(NCH - 1, 1)

    nc.compile()
    return nc


_NC_CACHE = None


def _get_program():
    global _NC_CACHE
    if _NC_CACHE is None:
        _NC_CACHE = build_program()
    return _NC_CACHE


def kernel(**inputs) -> np.ndarray:
    from concourse.bass_utils import run_bass_kernel_spmd

    x = np.ascontiguousarray(np.asarray(inputs["x"], dtype=np.float32))
    B = x.shape[0]
    w_q = np.ascontiguousarray(np.asarray(inputs["w_q"], dtype=np.float32))
    w_k = np.ascontiguousarray(np.asarray(inputs["w_k"], dtype=np.float32))
    w_v = np.ascontiguousarray(np.asarray(inputs["w_v"], dtype=np.float32))
    w_o = np.ascontiguousarray(np.asarray(inputs["w_o"], dtype=np.float32))
    gamma = np.asarray(inputs["gamma"], dtype=np.float32).reshape(1, 1)

    nc = _get_program()
    in_maps = [
        {
            "x": x[b].reshape(C, NQ),
            "w_q": w_q, "w_k": w_k, "w_v": w_v, "w_o": w_o,
            "gamma": gamma,
        }
        for b in range(B)
    ]
    res = run_bass_kernel_spmd(nc, in_maps, core_ids=list(range(B)))
    out = np.stack([res.results[b]["out"].reshape(C, LL, LL, LL)
                    for b in range(B)])
    return out.astype(np.float32)


if __name__ == "__main__":
    nc = build_program()
    print("program built OK")
